# revision 22
# baseline (speedup 1.0000x reference)
"""Bass/Tile kernel for multi-head self-attention on 8 TRN2 NeuronCores.

Problem: B=16, S=1024, D=768, H=12, head_dim=64, fp32 in/out.
Strategy: data parallel over batch (2 batch items per core, no collectives).

Per-core layout (bf16 matmul operands, fp32 accumulation):
  - x is PE-transposed to xT [c, t] (feature-major); weights pre-transposed
    to wT [c_in, c_out] once.
  - qT, kT computed feature-major [o, t]; v computed token-major [t, o] and
    stored with a ones column appended per head (v_aug), so the P@V matmul
    also produces softmax denominators in its last output row.
  - scoresT [j, i] = kT_h.T @ qT_h (contraction over head_dim=64); exp on
    ScalarE straight out of PSUM (4 banks per op, FD=2048) with the
    1/sqrt(hd) scale folded into the activation.
  - P@V accumulates over the 8 key tiles into psum [65, 512]; row 64 holds
    sum_j exp(scores). Unnormalized output is evacuated to attn_T.
  - Per batch: one DVE reciprocal over all heads' sums [12, 1024], then a
    ones-matmul broadcasts recip across 64 partitions and a DVE multiply
    normalizes attn_T in place.
  - Final projection reuses attn_T as lhsT to produce natural [t, o] output
    tiles which DMA straight to DRAM.
"""

import contextlib
import threading

import numpy as np

import concourse.bass as bass
import concourse.tile as tile
from concourse import bacc, mybir
from concourse.bass_utils import run_bass_kernel_spmd
from concourse.masks import make_identity

N_CORES = 8
B, S, D = 16, 1024, 768
H, HD = 12, 64
BPC = B // N_CORES  # batch items per core

P = 128
CC = D // P          # 6 feature chunks of 128
TN = 512             # matmul moving free dim
NT = S // TN         # 2 token chunks of 512
TT = S // P          # 8 token tiles of 128
JT = S // P          # 8 key tiles of 128
HPC = P // HD        # 2 heads per feature chunk

F32 = mybir.dt.float32
BF16 = mybir.dt.bfloat16
F32R = mybir.dt.float32r

AF = mybir.ActivationFunctionType
ALU = mybir.AluOpType


def build_kernel(tc: "tile.TileContext", outs, ins):
    nc = tc.nc
    x_d = ins["x"]
    out_d = outs["out"]

    ctx = contextlib.ExitStack()
    with ctx:
        const = ctx.enter_context(tc.tile_pool(name="const", bufs=1))
        wpool = ctx.enter_context(tc.tile_pool(name="wts", bufs=1))
        iop = ctx.enter_context(tc.tile_pool(name="iop", bufs=3))
        work = ctx.enter_context(tc.tile_pool(name="work", bufs=1))
        probs_pool = ctx.enter_context(tc.tile_pool(name="probs", bufs=1))
        small = ctx.enter_context(tc.tile_pool(name="small", bufs=2))
        psum_mm = ctx.enter_context(tc.tile_pool(name="psum_mm", bufs=4, space="PSUM"))
        psum_sc = ctx.enter_context(tc.tile_pool(name="psum_sc", bufs=1, space="PSUM"))

        # ---- one-time constants ----
        ident = const.tile([P, P], BF16)
        make_identity(nc, ident)

        ones_row = const.tile([1, P], F32)
        nc.vector.memset(ones_row, 1.0)
        ones_r = const.tile([1, P], F32R)
        nc.vector.tensor_copy(ones_r, ones_row)

        # head-selector matrix: sel[k, h*64+j] = (k == h); used to broadcast
        # recip[h, :] across 64 output partitions with a K=12 matmul.
        # Constant, supplied by the host (DVE can't memset at partition>0).
        sel_f = const.tile([H, H * HD], F32)
        nc.sync.dma_start(sel_f, ins["sel"])
        sel = const.tile([H, H * HD], BF16)
        nc.vector.tensor_copy(sel, sel_f)

        # q/k biases laid out per-partition: [p, oc] = b[oc*128 + p]
        bq = const.tile([P, CC], F32)
        bk = const.tile([P, CC], F32)
        with nc.allow_non_contiguous_dma(reason="tiny bias load"):
            nc.sync.dma_start(bq, ins["wq_b"].rearrange("(oc p) -> p oc", p=P))
            nc.sync.dma_start(bk, ins["wk_b"].rearrange("(oc p) -> p oc", p=P))

        # v/out biases broadcast along partitions: [128, 768] via ones-matmul
        bias_bc = {}
        for name in ("wv_b", "wo_b"):
            brow_f = small.tile([1, D], F32, name=f"{name}_rowf", tag="brow_f", bufs=1)
            nc.sync.dma_start(brow_f, ins[name][None, :])
            brow = small.tile([1, D], F32R, name=f"{name}_row", tag="brow_r", bufs=1)
            nc.vector.tensor_copy(brow, brow_f)
            bc = const.tile([P, D], F32, name=f"{name}_bc")
            for n0 in range(0, D, TN):
                nsz = min(TN, D - n0)
                pb = psum_mm.tile([P, TN], F32, tag="pmm")
                nc.tensor.matmul(
                    pb[:, :nsz],
                    ones_r,
                    brow[:, n0 : n0 + nsz],
                    start=True,
                    stop=True,
                )
                nc.vector.tensor_copy(bc[:, n0 : n0 + nsz], pb[:, :nsz])
            bias_bc[name] = bc

        def x_transpose(b):
            # x transpose: xT [c(6 chunks of 128), t=1024] bf16 via xbar DMA
            xT = work.tile([P, CC, S], BF16, tag="xT", name="xT", bufs=2)
            for tt in range(TT):
                xf = iop.tile([P, D], F32, tag="nat_f", name="xf")
                nc.sync.dma_start(xf, x_d[b, tt * P : (tt + 1) * P, :])
                xb = iop.tile([P, D], BF16, tag="nat_b", name="xb")
                nc.vector.tensor_copy(xb, xf)
                for cc in range(CC):
                    nc.sync.dma_start_transpose(
                        xT[:, cc, tt * P : (tt + 1) * P],
                        xb[:, cc * P : (cc + 1) * P],
                    )
            return xT

        # batch 0's x transposes fill the PE while weight DMAs stream in
        xT0 = x_transpose(0)

        # ---- weight transposes: wT[c_in chunk, c_out] bf16 ----
        wT = {}
        for name in ("wv_w", "wq_w", "wk_w", "wo_w"):
            wt = wpool.tile([P, CC, D], BF16, name=f"{name}_T")
            for oc in range(CC):
                wnat_f = iop.tile([P, D], F32, tag="nat_f")
                nc.sync.dma_start(wnat_f, ins[name][oc * P : (oc + 1) * P, :])
                wnat = iop.tile([P, D], BF16, tag="nat_b")
                nc.vector.tensor_copy(wnat, wnat_f)
                for cc in range(CC):
                    nc.sync.dma_start_transpose(
                        wt[:, cc, oc * P : (oc + 1) * P],
                        wnat[:, cc * P : (cc + 1) * P],
                    )
            wT[name] = wt

        # ---- per-batch ----
        for b in range(BPC):
            xT = xT0 if b == 0 else x_transpose(b)

            # qT, kT: [o(6 chunks), t] bf16, bias fused in ACT evacuation.
            # Emitted per-chunk so chunks oc>=2 can interleave with the
            # attention pair loop (keeps TensorE dense while ACT runs exp).
            qT = work.tile([P, CC, S], BF16, tag="qT")
            kT = work.tile([P, CC, S], BF16, tag="kT")

            def proj_qk_chunk(oc, xT=xT, qT=qT, kT=kT):
                for dst, wname, bap in ((qT, "wq_w", bq), (kT, "wk_w", bk)):
                    wt = wT[wname]
                    for nt in range(NT):
                        pq = psum_mm.tile([P, TN], F32, tag="pmm", name="pq")
                        for cc in range(CC):
                            nc.tensor.matmul(
                                pq,
                                wt[:, cc, oc * P : (oc + 1) * P],
                                xT[:, cc, nt * TN : (nt + 1) * TN],
                                start=(cc == 0),
                                stop=(cc == CC - 1),
                            )
                        nc.vector.tensor_tensor(
                            dst[:, oc, nt * TN : (nt + 1) * TN],
                            pq,
                            bap[:, oc : oc + 1].to_broadcast((P, TN)),
                            ALU.add,
                        )

            # v_aug: [t-tile partitions, t-tile(8), h(12), 65] with ones col
            v_aug = work.tile([P, TT, H, HD + 1], BF16, tag="v_aug")
            nc.vector.memset(v_aug[:, :, :, HD : HD + 1], 1.0)
            wv = wT["wv_w"]
            for mt in range(TT):
                for n0 in range(0, D, TN):
                    nsz = min(TN, D - n0)
                    pv = psum_mm.tile([P, TN], F32, tag="pmm")
                    for cc in range(CC):
                        nc.tensor.matmul(
                            pv[:, :nsz],
                            xT[:, cc, mt * P : (mt + 1) * P],
                            wv[:, cc, n0 : n0 + nsz],
                            start=(cc == 0),
                            stop=(cc == CC - 1),
                        )
                    h0 = n0 // HD
                    nh = nsz // HD
                    nc.vector.tensor_tensor(
                        v_aug[:, mt, h0 : h0 + nh, 0:HD],
                        pv[:, :nsz].rearrange("p (h d) -> p h d", d=HD),
                        bias_bc["wv_b"][:, n0 : n0 + nsz].rearrange(
                            "p (h d) -> p h d", d=HD
                        ),
                        ALU.add,
                    )

            # ---- attention pass A: QK^T, exp, P@V, sums; unnormalized ----
            attn_T = work.tile([P, CC, S], BF16, tag="attn_T")
            rsum = small.tile([H, S], F32, tag="rsum", bufs=1)
            recip = small.tile([H, S], F32, tag="recip", bufs=1)

            proj_qk_chunk(0)
            proj_qk_chunk(1)

            for pair in range(H // 2):
                stage_s = small.tile([P, 2, S], F32, tag="stage_s", bufs=1)
                # QK^T packed per head pair: the two heads of a chunk sit on
                # row groups 0-1 / 2-3, so their matmuls run concurrently in
                # the PE array. 2-bank score tiles double-buffered keep ACT's
                # exp stream dense; both ic chunks' QK+exp are emitted before
                # the P@V matmuls so exp(ic1) overlaps PV(ic0).
                probsT_ic = []
                for ic in range(NT):
                    probsT = probs_pool.tile(
                        [P, 2, JT, TN], BF16, tag="probsT", bufs=2
                    )
                    probsT_ic.append(probsT)
                    for jt in range(JT):
                        sq = psum_sc.tile([P, 2, TN], F32, tag="sq", bufs=2)
                        for hi in range(2):
                            hp = hi * HD
                            nc.tensor.matmul(
                                sq[:, hi],
                                kT[hp : hp + HD, pair, jt * P : (jt + 1) * P],
                                qT[hp : hp + HD, pair, ic * TN : (ic + 1) * TN],
                                start=True,
                                stop=True,
                            )
                        nc.scalar.activation(
                            probsT[:, :, jt, :],
                            sq,
                            AF.Exp,
                            scale=float(1.0 / np.sqrt(HD)),
                        )

                for ic in range(NT):
                    probsT = probsT_ic[ic]
                    for hi in range(2):
                        h = pair * 2 + hi
                        hc = h // HPC
                        hp = (h % HPC) * HD
                        po = psum_mm.tile([P, TN], F32, tag="pmm")
                        for jt in range(JT):
                            nc.tensor.matmul(
                                po[: HD + 1, :],
                                v_aug[:, jt, h, :],
                                probsT[:, hi, jt, :],
                                start=(jt == 0),
                                stop=(jt == JT - 1),
                            )
                        nc.vector.tensor_copy(
                            stage_s[HD : HD + 1, hi, ic * TN : (ic + 1) * TN],
                            po[HD : HD + 1, :],
                        )
                        if hp == 0:
                            nc.vector.tensor_copy(
                                attn_T[0:HD, hc, ic * TN : (ic + 1) * TN],
                                po[:HD, :],
                            )
                        else:
                            # DVE lanes can't cross partitions; bounce via DMA
                            tmp = small.tile([HD, TN], BF16, tag="odd_tmp")
                            nc.vector.tensor_copy(tmp, po[:HD, :])
                            nc.sync.dma_start(
                                attn_T[HD:P, hc, ic * TN : (ic + 1) * TN], tmp
                            )

                if pair + 2 < CC:
                    proj_qk_chunk(pair + 2)

                for hi in range(2):
                    nc.sync.dma_start(
                        rsum[pair * 2 + hi : pair * 2 + hi + 1, :],
                        stage_s[HD : HD + 1, hi, :],
                    )

            # ---- per batch: reciprocal over all heads' sums ----
            nc.vector.reciprocal_approx_fast(recip, rsum)
            recip_r = small.tile([H, S], BF16, tag="recip_r", bufs=1)
            nc.vector.tensor_copy(recip_r, recip)

            # ---- attention pass B: broadcast recip, normalize in place.
            # One matmul covers both heads of a feature chunk: sel[k, c] =
            # (k == c//64), so pb[c_local, i] = recip[head(c), i].
            for hc in range(CC):
                for ic in range(NT):
                    pb = psum_mm.tile([P, TN], F32, tag="pmm")
                    nc.tensor.matmul(
                        pb,
                        sel[:, hc * P : (hc + 1) * P],
                        recip_r[:, ic * TN : (ic + 1) * TN],
                        start=True,
                        stop=True,
                    )
                    rb = small.tile([P, TN], BF16, tag="rb")
                    nc.scalar.copy(rb, pb)
                    sl = attn_T[:, hc, ic * TN : (ic + 1) * TN]
                    nc.vector.tensor_tensor(sl, sl, rb, ALU.mult)

            # ---- output projection: natural [t, o] ----
            wo = wT["wo_w"]
            for mt in range(TT):
                out_sb = iop.tile([P, D], F32, tag="out_sb")
                for n0 in range(0, D, TN):
                    nsz = min(TN, D - n0)
                    pf = psum_mm.tile([P, TN], F32, tag="pmm")
                    for cc in range(CC):
                        nc.tensor.matmul(
                            pf[:, :nsz],
                            attn_T[:, cc, mt * P : (mt + 1) * P],
                            wo[:, cc, n0 : n0 + nsz],
                            start=(cc == 0),
                            stop=(cc == CC - 1),
                        )
                    nc.vector.tensor_tensor(
                        out_sb[:, n0 : n0 + nsz],
                        pf[:, :nsz],
                        bias_bc["wo_b"][:, n0 : n0 + nsz],
                        ALU.add,
                    )
                nc.sync.dma_start(out_d[b, mt * P : (mt + 1) * P, :], out_sb)


_BUILD_LOCK = threading.Lock()
_BUILT = {}


def build():
    with _BUILD_LOCK:
        if "nc" in _BUILT:
            return _BUILT["nc"]
        nc = bacc.Bacc(
            "TRN2",
            target_bir_lowering=False,
            debug=False,
            enable_asserts=True,
            num_devices=N_CORES,
        )
        ins = {
            "x": nc.dram_tensor("x", [BPC, S, D], F32, kind="ExternalInput").ap(),
            "sel": nc.dram_tensor(
                "sel", [H, H * HD], F32, kind="ExternalInput"
            ).ap(),
        }
        for w in ("wq_w", "wk_w", "wv_w", "wo_w"):
            ins[w] = nc.dram_tensor(w, [D, D], F32, kind="ExternalInput").ap()
        for bn in ("wq_b", "wk_b", "wv_b", "wo_b"):
            ins[bn] = nc.dram_tensor(bn, [D], F32, kind="ExternalInput").ap()
        outs = {
            "out": nc.dram_tensor(
                "out", [BPC, S, D], F32, kind="ExternalOutput"
            ).ap()
        }
        with tile.TileContext(nc) as tc:
            build_kernel(tc, outs, ins)
        nc.compile()
        _BUILT["nc"] = nc
        return nc


def make_in_maps(inputs):
    x = np.ascontiguousarray(np.asarray(inputs["x"], dtype=np.float32))
    shared = {
        k: np.ascontiguousarray(np.asarray(inputs[k], dtype=np.float32))
        for k in (
            "wq_w", "wq_b", "wk_w", "wk_b", "wv_w", "wv_b", "wo_w", "wo_b",
        )
    }
    sel = np.kron(np.eye(H, dtype=np.float32), np.ones((1, HD), np.float32))
    in_maps = []
    for c in range(N_CORES):
        m = {"x": x[c * BPC : (c + 1) * BPC], "sel": sel}
        m.update(shared)
        in_maps.append(m)
    return in_maps


def _ensure_profile_hook():
    """Install the axon NTFF profile hook shim if the container lacks it."""
    try:
        from antenv.axon_hooks import get_axon_ntff_profile_hook  # noqa: F401

        return
    except ImportError:
        pass
    try:
        import sys
        import types

        from trn_agent_boot.trn_boot import _ntff_profile_via_ctypes

        state = {"h": None}
        mod = types.ModuleType("antenv.axon_hooks")
        mod.set_axon_ntff_profile_hook = lambda h: state.__setitem__("h", h)
        mod.get_axon_ntff_profile_hook = lambda: state["h"]
        sys.modules["antenv.axon_hooks"] = mod
        mod.set_axon_ntff_profile_hook(
            _ntff_profile_via_ctypes("/opt/axon/libaxon_pjrt.so")
        )

        import concourse.bass_utils as bu

        orig_upload = bu.upload_artifacts

        def _safe_upload(d, *a, **k):
            try:
                return orig_upload(d, *a, **k)
            except Exception:
                return str(d)

        bu.upload_artifacts = _safe_upload
    except Exception:
        pass


def run(inputs, trace=False, **kwargs):
    """Returns (full_output [B,S,D] f32, BassKernelResults)."""
    if trace:
        _ensure_profile_hook()
    nc = build()
    res = run_bass_kernel_spmd(
        nc, make_in_maps(inputs), core_ids=list(range(N_CORES)),
        trace=trace, **kwargs,
    )
    out = np.concatenate([res.results[c]["out"] for c in range(N_CORES)], axis=0)
    return out, res


def kernel(**inputs):
    out, _ = run(inputs, trace=False)
    return out


# revision 23
# speedup vs baseline: 1.8965x; 1.8965x over previous
"""Bass/Tile kernel for multi-head self-attention on 8 TRN2 NeuronCores.

Problem: B=16, S=1024, D=768, H=12, head_dim=64, fp32 in/out.
Strategy: data parallel over batch (2 batch items per core, no collectives).

Per-core layout (bf16 matmul operands, fp32 accumulation):
  - x is PE-transposed to xT [c, t] (feature-major); weights pre-transposed
    to wT [c_in, c_out] once.
  - qT, kT computed feature-major [o, t]; v computed token-major [t, o] and
    stored with a ones column appended per head (v_aug), so the P@V matmul
    also produces softmax denominators in its last output row.
  - scoresT [j, i] = kT_h.T @ qT_h (contraction over head_dim=64); exp on
    ScalarE straight out of PSUM (4 banks per op, FD=2048) with the
    1/sqrt(hd) scale folded into the activation.
  - P@V accumulates over the 8 key tiles into psum [65, 512]; row 64 holds
    sum_j exp(scores). Unnormalized output is evacuated to attn_T.
  - Per batch: one DVE reciprocal over all heads' sums [12, 1024], then a
    ones-matmul broadcasts recip across 64 partitions and a DVE multiply
    normalizes attn_T in place.
  - Final projection reuses attn_T as lhsT to produce natural [t, o] output
    tiles which DMA straight to DRAM.
"""

import contextlib
import threading

import numpy as np

import concourse.bass as bass
import concourse.tile as tile
from concourse import bacc, mybir
from concourse.bass_utils import run_bass_kernel_spmd
from concourse.masks import make_identity

N_CORES = 8
B, S, D = 16, 1024, 768
H, HD = 12, 64
BPC = B // N_CORES  # batch items per core

P = 128
CC = D // P          # 6 feature chunks of 128
TN = 512             # matmul moving free dim
NT = S // TN         # 2 token chunks of 512
TT = S // P          # 8 token tiles of 128
JT = S // P          # 8 key tiles of 128
HPC = P // HD        # 2 heads per feature chunk

F32 = mybir.dt.float32
BF16 = mybir.dt.bfloat16
F32R = mybir.dt.float32r

AF = mybir.ActivationFunctionType
ALU = mybir.AluOpType


def build_kernel(tc: "tile.TileContext", outs, ins):
    nc = tc.nc
    x_d = ins["x"]
    out_d = outs["out"]

    ctx = contextlib.ExitStack()
    with ctx:
        const = ctx.enter_context(tc.tile_pool(name="const", bufs=1))
        wpool = ctx.enter_context(tc.tile_pool(name="wts", bufs=1))
        iop = ctx.enter_context(tc.tile_pool(name="iop", bufs=3))
        work = ctx.enter_context(tc.tile_pool(name="work", bufs=1))
        probs_pool = ctx.enter_context(tc.tile_pool(name="probs", bufs=1))
        small = ctx.enter_context(tc.tile_pool(name="small", bufs=2))
        psum_mm = ctx.enter_context(tc.tile_pool(name="psum_mm", bufs=3, space="PSUM"))
        psum_sc = ctx.enter_context(tc.tile_pool(name="psum_sc", bufs=1, space="PSUM"))

        # ---- one-time constants ----
        ident = const.tile([P, P], BF16)
        make_identity(nc, ident)

        ones_row = const.tile([1, P], F32)
        nc.vector.memset(ones_row, 1.0)
        ones_r = const.tile([1, P], F32R)
        nc.vector.tensor_copy(ones_r, ones_row)

        # head-selector matrix: sel[k, h*64+j] = (k == h); used to broadcast
        # recip[h, :] across 64 output partitions with a K=12 matmul.
        # Constant, supplied by the host (DVE can't memset at partition>0).
        sel_f = const.tile([H, H * HD], F32)
        nc.sync.dma_start(sel_f, ins["sel"])
        sel = const.tile([H, H * HD], BF16)
        nc.vector.tensor_copy(sel, sel_f)

        # q/k biases laid out per-partition: [p, oc] = b[oc*128 + p]
        bq = const.tile([P, CC], F32)
        bk = const.tile([P, CC], F32)
        with nc.allow_non_contiguous_dma(reason="tiny bias load"):
            nc.sync.dma_start(bq, ins["wq_b"].rearrange("(oc p) -> p oc", p=P))
            nc.sync.dma_start(bk, ins["wk_b"].rearrange("(oc p) -> p oc", p=P))

        # v/out biases broadcast along partitions: [128, 768] via ones-matmul
        bias_bc = {}
        for name in ("wv_b", "wo_b"):
            brow_f = small.tile([1, D], F32, name=f"{name}_rowf", tag="brow_f", bufs=1)
            nc.sync.dma_start(brow_f, ins[name][None, :])
            brow = small.tile([1, D], F32R, name=f"{name}_row", tag="brow_r", bufs=1)
            nc.vector.tensor_copy(brow, brow_f)
            bc = const.tile([P, D], F32, name=f"{name}_bc")
            for n0 in range(0, D, TN):
                nsz = min(TN, D - n0)
                pb = psum_mm.tile([P, TN], F32, tag="pmm")
                nc.tensor.matmul(
                    pb[:, :nsz],
                    ones_r,
                    brow[:, n0 : n0 + nsz],
                    start=True,
                    stop=True,
                )
                nc.vector.tensor_copy(bc[:, n0 : n0 + nsz], pb[:, :nsz])
            bias_bc[name] = bc

        def x_transpose(b):
            # x transpose: xT [c(6 chunks of 128), t=1024] bf16 via xbar DMA
            xT = work.tile([P, CC, S], BF16, tag="xT", name="xT", bufs=2)
            for tt in range(TT):
                xf = iop.tile([P, D], F32, tag="nat_f", name="xf")
                nc.sync.dma_start(xf, x_d[b, tt * P : (tt + 1) * P, :])
                xb = iop.tile([P, D], BF16, tag="nat_b", name="xb")
                nc.vector.tensor_copy(xb, xf)
                ptr = psum_mm.tile([P, CC, P], BF16, tag="ptr", bufs=1)
                for cc in range(CC):
                    nc.tensor.transpose(
                        ptr[:, cc], xb[:, cc * P : (cc + 1) * P], ident
                    )
                nc.vector.tensor_copy(xT[:, :, tt * P : (tt + 1) * P], ptr)
            return xT

        # batch 0's x transposes fill the PE while weight DMAs stream in
        xT0 = x_transpose(0)

        # ---- weight transposes: wT[c_in chunk, c_out] bf16 ----
        wT = {}
        for name in ("wv_w", "wq_w", "wk_w", "wo_w"):
            wt = wpool.tile([P, CC, D], BF16, name=f"{name}_T")
            for oc in range(CC):
                wnat_f = iop.tile([P, D], F32, tag="nat_f")
                nc.sync.dma_start(wnat_f, ins[name][oc * P : (oc + 1) * P, :])
                wnat = iop.tile([P, D], BF16, tag="nat_b")
                nc.vector.tensor_copy(wnat, wnat_f)
                ptr = psum_mm.tile([P, CC, P], BF16, tag="ptr", bufs=1)
                for cc in range(CC):
                    nc.tensor.transpose(
                        ptr[:, cc], wnat[:, cc * P : (cc + 1) * P], ident
                    )
                nc.vector.tensor_copy(wt[:, :, oc * P : (oc + 1) * P], ptr)
            wT[name] = wt

        # ---- per-batch ----
        for b in range(BPC):
            xT = xT0 if b == 0 else x_transpose(b)

            # qT, kT: [o(6 chunks), t] bf16, bias fused in ACT evacuation.
            # Emitted per-chunk so chunks oc>=2 can interleave with the
            # attention pair loop (keeps TensorE dense while ACT runs exp).
            qT = work.tile([P, CC, S], BF16, tag="qT")
            kT = work.tile([P, CC, S], BF16, tag="kT")

            def proj_qk_chunk(oc, xT=xT, qT=qT, kT=kT):
                for dst, wname, bap in ((qT, "wq_w", bq), (kT, "wk_w", bk)):
                    wt = wT[wname]
                    for nt in range(NT):
                        pq = psum_mm.tile([P, TN], F32, tag="pmm", name="pq")
                        for cc in range(CC):
                            nc.tensor.matmul(
                                pq,
                                wt[:, cc, oc * P : (oc + 1) * P],
                                xT[:, cc, nt * TN : (nt + 1) * TN],
                                start=(cc == 0),
                                stop=(cc == CC - 1),
                            )
                        nc.vector.tensor_tensor(
                            dst[:, oc, nt * TN : (nt + 1) * TN],
                            pq,
                            bap[:, oc : oc + 1].to_broadcast((P, TN)),
                            ALU.add,
                        )

            # v_aug: [t-tile partitions, t-tile(8), h(12), 65] with ones col
            v_aug = work.tile([P, TT, H, HD + 1], BF16, tag="v_aug")
            nc.vector.memset(v_aug[:, :, :, HD : HD + 1], 1.0)
            wv = wT["wv_w"]
            for mt in range(TT):
                for n0 in range(0, D, TN):
                    nsz = min(TN, D - n0)
                    pv = psum_mm.tile([P, TN], F32, tag="pmm")
                    for cc in range(CC):
                        nc.tensor.matmul(
                            pv[:, :nsz],
                            xT[:, cc, mt * P : (mt + 1) * P],
                            wv[:, cc, n0 : n0 + nsz],
                            start=(cc == 0),
                            stop=(cc == CC - 1),
                        )
                    h0 = n0 // HD
                    nh = nsz // HD
                    nc.vector.tensor_tensor(
                        v_aug[:, mt, h0 : h0 + nh, 0:HD],
                        pv[:, :nsz].rearrange("p (h d) -> p h d", d=HD),
                        bias_bc["wv_b"][:, n0 : n0 + nsz].rearrange(
                            "p (h d) -> p h d", d=HD
                        ),
                        ALU.add,
                    )

            # ---- attention pass A: QK^T, exp, P@V, sums; unnormalized ----
            attn_T = work.tile([P, CC, S], BF16, tag="attn_T")
            rsum = small.tile([H, S], F32, tag="rsum", bufs=1)
            recip = small.tile([H, S], F32, tag="recip", bufs=1)

            proj_qk_chunk(0)
            proj_qk_chunk(1)

            for pair in range(H // 2):
                stage_s = small.tile([P, 2, S], F32, tag="stage_s", bufs=1)
                # QK^T packed per head pair: the two heads of a chunk sit on
                # row groups 0-1 / 2-3, so their matmuls run concurrently in
                # the PE array. 2-bank score tiles double-buffered keep ACT's
                # exp stream dense; both ic chunks' QK+exp are emitted before
                # the P@V matmuls so exp(ic1) overlaps PV(ic0).
                probsT_ic = []
                for ic in range(NT):
                    probsT = probs_pool.tile(
                        [P, 2, JT, TN], BF16, tag="probsT", bufs=2
                    )
                    probsT_ic.append(probsT)
                    for jt in range(JT):
                        sq = psum_sc.tile([P, 2, TN], F32, tag="sq", bufs=2)
                        for hi in range(2):
                            hp = hi * HD
                            nc.tensor.matmul(
                                sq[:, hi],
                                kT[hp : hp + HD, pair, jt * P : (jt + 1) * P],
                                qT[hp : hp + HD, pair, ic * TN : (ic + 1) * TN],
                                start=True,
                                stop=True,
                            )
                        nc.scalar.activation(
                            probsT[:, :, jt, :],
                            sq,
                            AF.Exp,
                            scale=float(1.0 / np.sqrt(HD)),
                        )

                for ic in range(NT):
                    probsT = probsT_ic[ic]
                    for hi in range(2):
                        h = pair * 2 + hi
                        hc = h // HPC
                        hp = (h % HPC) * HD
                        po = psum_mm.tile([P, TN], F32, tag="pmm")
                        for jt in range(JT):
                            nc.tensor.matmul(
                                po[: HD + 1, :],
                                v_aug[:, jt, h, :],
                                probsT[:, hi, jt, :],
                                start=(jt == 0),
                                stop=(jt == JT - 1),
                            )
                        nc.vector.tensor_copy(
                            stage_s[HD : HD + 1, hi, ic * TN : (ic + 1) * TN],
                            po[HD : HD + 1, :],
                        )
                        if hp == 0:
                            nc.vector.tensor_copy(
                                attn_T[0:HD, hc, ic * TN : (ic + 1) * TN],
                                po[:HD, :],
                            )
                        else:
                            # DVE lanes can't cross partitions; bounce via DMA
                            tmp = small.tile([HD, TN], BF16, tag="odd_tmp")
                            nc.vector.tensor_copy(tmp, po[:HD, :])
                            nc.sync.dma_start(
                                attn_T[HD:P, hc, ic * TN : (ic + 1) * TN], tmp
                            )

                if pair + 2 < CC:
                    proj_qk_chunk(pair + 2)

                for hi in range(2):
                    nc.sync.dma_start(
                        rsum[pair * 2 + hi : pair * 2 + hi + 1, :],
                        stage_s[HD : HD + 1, hi, :],
                    )

            # ---- per batch: reciprocal over all heads' sums ----
            nc.vector.reciprocal_approx_fast(recip, rsum)
            recip_r = small.tile([H, S], BF16, tag="recip_r", bufs=1)
            nc.vector.tensor_copy(recip_r, recip)

            # ---- attention pass B: broadcast recip, normalize in place.
            # One matmul covers both heads of a feature chunk: sel[k, c] =
            # (k == c//64), so pb[c_local, i] = recip[head(c), i].
            for hc in range(CC):
                for ic in range(NT):
                    pb = psum_mm.tile([P, TN], F32, tag="pmm")
                    nc.tensor.matmul(
                        pb,
                        sel[:, hc * P : (hc + 1) * P],
                        recip_r[:, ic * TN : (ic + 1) * TN],
                        start=True,
                        stop=True,
                    )
                    rb = small.tile([P, TN], BF16, tag="rb")
                    nc.scalar.copy(rb, pb)
                    sl = attn_T[:, hc, ic * TN : (ic + 1) * TN]
                    nc.vector.tensor_tensor(sl, sl, rb, ALU.mult)

            # ---- output projection: natural [t, o] ----
            wo = wT["wo_w"]
            for mt in range(TT):
                out_sb = iop.tile([P, D], F32, tag="out_sb")
                for n0 in range(0, D, TN):
                    nsz = min(TN, D - n0)
                    pf = psum_mm.tile([P, TN], F32, tag="pmm")
                    for cc in range(CC):
                        nc.tensor.matmul(
                            pf[:, :nsz],
                            attn_T[:, cc, mt * P : (mt + 1) * P],
                            wo[:, cc, n0 : n0 + nsz],
                            start=(cc == 0),
                            stop=(cc == CC - 1),
                        )
                    nc.vector.tensor_tensor(
                        out_sb[:, n0 : n0 + nsz],
                        pf[:, :nsz],
                        bias_bc["wo_b"][:, n0 : n0 + nsz],
                        ALU.add,
                    )
                nc.sync.dma_start(out_d[b, mt * P : (mt + 1) * P, :], out_sb)


_BUILD_LOCK = threading.Lock()
_BUILT = {}


def build():
    with _BUILD_LOCK:
        if "nc" in _BUILT:
            return _BUILT["nc"]
        nc = bacc.Bacc(
            "TRN2",
            target_bir_lowering=False,
            debug=False,
            enable_asserts=True,
            num_devices=N_CORES,
        )
        ins = {
            "x": nc.dram_tensor("x", [BPC, S, D], F32, kind="ExternalInput").ap(),
            "sel": nc.dram_tensor(
                "sel", [H, H * HD], F32, kind="ExternalInput"
            ).ap(),
        }
        for w in ("wq_w", "wk_w", "wv_w", "wo_w"):
            ins[w] = nc.dram_tensor(w, [D, D], F32, kind="ExternalInput").ap()
        for bn in ("wq_b", "wk_b", "wv_b", "wo_b"):
            ins[bn] = nc.dram_tensor(bn, [D], F32, kind="ExternalInput").ap()
        outs = {
            "out": nc.dram_tensor(
                "out", [BPC, S, D], F32, kind="ExternalOutput"
            ).ap()
        }
        with tile.TileContext(nc) as tc:
            build_kernel(tc, outs, ins)
        nc.compile()
        _BUILT["nc"] = nc
        return nc


def make_in_maps(inputs):
    x = np.ascontiguousarray(np.asarray(inputs["x"], dtype=np.float32))
    shared = {
        k: np.ascontiguousarray(np.asarray(inputs[k], dtype=np.float32))
        for k in (
            "wq_w", "wq_b", "wk_w", "wk_b", "wv_w", "wv_b", "wo_w", "wo_b",
        )
    }
    sel = np.kron(np.eye(H, dtype=np.float32), np.ones((1, HD), np.float32))
    in_maps = []
    for c in range(N_CORES):
        m = {"x": x[c * BPC : (c + 1) * BPC], "sel": sel}
        m.update(shared)
        in_maps.append(m)
    return in_maps


def _ensure_profile_hook():
    """Install the axon NTFF profile hook shim if the container lacks it."""
    try:
        from antenv.axon_hooks import get_axon_ntff_profile_hook  # noqa: F401

        return
    except ImportError:
        pass
    try:
        import sys
        import types

        from trn_agent_boot.trn_boot import _ntff_profile_via_ctypes

        state = {"h": None}
        mod = types.ModuleType("antenv.axon_hooks")
        mod.set_axon_ntff_profile_hook = lambda h: state.__setitem__("h", h)
        mod.get_axon_ntff_profile_hook = lambda: state["h"]
        sys.modules["antenv.axon_hooks"] = mod
        mod.set_axon_ntff_profile_hook(
            _ntff_profile_via_ctypes("/opt/axon/libaxon_pjrt.so")
        )

        import concourse.bass_utils as bu

        orig_upload = bu.upload_artifacts

        def _safe_upload(d, *a, **k):
            try:
                return orig_upload(d, *a, **k)
            except Exception:
                return str(d)

        bu.upload_artifacts = _safe_upload
    except Exception:
        pass


def run(inputs, trace=False, **kwargs):
    """Returns (full_output [B,S,D] f32, BassKernelResults)."""
    if trace:
        _ensure_profile_hook()
    nc = build()
    res = run_bass_kernel_spmd(
        nc, make_in_maps(inputs), core_ids=list(range(N_CORES)),
        trace=trace, **kwargs,
    )
    out = np.concatenate([res.results[c]["out"] for c in range(N_CORES)], axis=0)
    return out, res


def kernel(**inputs):
    out, _ = run(inputs, trace=False)
    return out
